# revision 11
# baseline (speedup 1.0000x reference)
"""GIN (3-layer) Trainium2 Bass kernel, 8-core SPMD.

Sharding: nodes (and their incident edges, by dst) are partitioned across the
8 cores; segment_sum is computed locally per dst shard; node features are
exchanged between layers with an AllGather; MLP weights are replicated.

v2 (fp8 + prepared gathers):
  - the gather path (x / h rows pulled per edge, and the one-hot selector S)
    runs in fp8e4 (TRN e4m3, max 240): halves HBM gather traffic, selector
    traffic, and the inter-layer AllGather size. MLP stays bf16, residual h
    stays fp32.
  - indirect gathers use SWDGE prepare_only + trigger_dma: descriptor
    generation for layer l+1's gathers runs on the Q7 during layer l's
    compute; the trigger (which carries the RAW dep on the AllGather output)
    fires them the moment the AllGather lands. One gather per dst block.
  - agg matmul: per 128-edge chunk, the fp8 one-hot selector S is the
    stationary operand, gathered fp8 rows are moving; chunks accumulate in
    PSUM -> agg[node, feat]; transposed on the PE and added to resident fp32
    h^T. The 2-layer MLP runs feature-major in bf16 with fused bias+ReLU on
    the scalar engine.
"""

import os
import sys
from contextlib import ExitStack

import numpy as np

for _p in ("/opt/trn_rl_repo", "/root/.axon_site/_ro/trn_rl_repo"):
    if os.path.isdir(_p) and _p not in sys.path:
        sys.path.append(_p)

import ml_dtypes

N_NODES = 10000
N_EDGES = 160000
D = 512
N_LAYERS = 3
CORES = 8
SHARD = N_NODES // CORES          # 1250 nodes per core
PADS = 1280                       # padded shard (multiple of 128)
PADN = CORES * PADS               # padded full node count (10240)
NB = PADS // 128                  # dst blocks per core (10)

BF16 = ml_dtypes.bfloat16
F8 = ml_dtypes.float8_e4m3        # TRN fp8e4 (e4m3, max normal 240)

# Results of the last kernel() call (BassKernelResults) for the test harness.
LAST_RESULTS = None


def _prep_host(x, edge_index, Ws, bs):
    """Per-core input maps + per-block chunk counts (uniform across cores)."""
    x = np.asarray(x, np.float32)
    src = np.asarray(edge_index[0], np.int64)
    dst = np.asarray(edge_index[1], np.int64)
    Ws = np.asarray(Ws, np.float32)
    bs = np.asarray(bs, np.float32)

    # Padded gather row index for every edge's source node.
    gidx_all = (src // SHARD) * PADS + (src % SHARD)

    owner = dst // SHARD
    li = dst % SHARD
    blk = li // 128
    slot = li - blk * 128

    # Per (core, block) unique-src counts (post-dedup) set the chunk counts.
    key = (owner * NB + blk) * PADN + gidx_all
    ucnt = np.zeros(CORES * NB, np.int64)
    kb = np.unique(key) // PADN
    np.add.at(ucnt, kb, 1)
    ucnt = ucnt.reshape(CORES, NB)
    C_list = [max(1, int(-(-ucnt[:, b].max() // 128))) for b in range(NB)]
    CMAX = max(C_list)

    # Full padded x in fp8 (gather source for layer 0), shared by all cores.
    xg_pad = np.zeros((PADN, D), F8)
    for o in range(CORES):
        xg_pad[o * PADS:o * PADS + SHARD] = x[o * SHARD:(o + 1) * SHARD].astype(F8)

    Wd = np.ascontiguousarray(Ws.reshape(2 * N_LAYERS, D, D).astype(BF16))
    bT = np.ascontiguousarray(
        bs.reshape(2 * N_LAYERS, 4, 128).transpose(2, 0, 1).reshape(128, 8 * N_LAYERS))
    ident = np.eye(128, dtype=np.float32)

    order = np.lexsort((blk, owner))  # edges grouped by (owner, block)
    e_sorted = order
    bounds = np.searchsorted(owner[order] * NB + blk[order], np.arange(CORES * NB + 1))

    in_maps = []
    for c in range(CORES):
        Sd = np.zeros((NB, 128, CMAX * 128), F8)
        idxd = np.zeros((128, NB * CMAX * 8), np.int16)
        for b in range(NB):
            C = C_list[b]
            lo, hi = bounds[c * NB + b], bounds[c * NB + b + 1]
            e = e_sorted[lo:hi]
            # Deduplicate src nodes within the block; S carries multiplicity.
            uniq, inv = np.unique(gidx_all[e], return_inverse=True)
            n = len(uniq)
            glist = np.zeros(C * 128, np.int16)
            glist[:n] = uniq.astype(np.int16)
            np.add.at(Sd[b], (inv % 128, (inv // 128) * 128 + slot[e]), 1.0)
            w = glist.reshape(C * 8, 16).T  # w[p, s] = glist[s*16 + p]
            idxd[:, b * CMAX * 8:b * CMAX * 8 + C * 8] = np.tile(w, (8, 1))
        xT_own = np.zeros((D, PADS), np.float32)
        xT_own[:, :SHARD] = x[c * SHARD:(c + 1) * SHARD].T
        in_maps.append({
            "xg": xg_pad,
            "xT": xT_own,
            "Wd": Wd,
            "bT": bT,
            "ident": ident,
            "Sd": Sd,
            "idxd": idxd,
        })
    return in_maps, C_list, CMAX


def build_program(C_list, CMAX):
    import concourse.bacc as bacc
    import concourse.bass as bass
    import concourse.mybir as mybir
    import concourse.tile as tile

    dt = mybir.dt
    f32, bf16, i16, f8 = dt.float32, dt.bfloat16, dt.int16, dt.float8e4
    AF = mybir.ActivationFunctionType

    nc = bacc.Bacc("TRN2", target_bir_lowering=False, debug=False,
                   enable_asserts=False, num_devices=CORES, num_swdge_queues=4)

    xg = nc.dram_tensor("xg", [PADN, D], f8, kind="ExternalInput")
    xT = nc.dram_tensor("xT", [D, PADS], f32, kind="ExternalInput")
    Wd = nc.dram_tensor("Wd", [2 * N_LAYERS, D, D], bf16, kind="ExternalInput")
    bTd = nc.dram_tensor("bT", [128, 8 * N_LAYERS], f32, kind="ExternalInput")
    identd = nc.dram_tensor("ident", [128, 128], f32, kind="ExternalInput")
    Sd = nc.dram_tensor("Sd", [NB, 128, CMAX * 128], f8, kind="ExternalInput")
    idxd = nc.dram_tensor("idxd", [128, NB * CMAX * 8], i16, kind="ExternalInput")
    outTd = nc.dram_tensor("outT", [D, PADS], f32, kind="ExternalOutput")

    NCHUNK = [(0, 512), (512, 512), (1024, PADS - 1024)]  # node-dim tiles for MLP

    with tile.TileContext(nc) as tc, ExitStack() as ctx:
        p_const = ctx.enter_context(tc.tile_pool(name="const", bufs=1))
        p_big = ctx.enter_context(tc.tile_pool(name="big", bufs=1))
        p_g = ctx.enter_context(tc.tile_pool(name="gth", bufs=NB))
        p_s = ctx.enter_context(tc.tile_pool(name="sel", bufs=4))
        p_aggn = ctx.enter_context(tc.tile_pool(name="aggn", bufs=3))
        p_w = ctx.enter_context(tc.tile_pool(name="wts", bufs=2))
        p_hbf = ctx.enter_context(tc.tile_pool(name="hbf", bufs=2))
        p_aggps = ctx.enter_context(tc.tile_pool(name="aggps", bufs=2, space="PSUM"))
        p_tps = ctx.enter_context(tc.tile_pool(name="tps", bufs=4, space="PSUM"))
        p_mlpps = ctx.enter_context(tc.tile_pool(name="mlpps", bufs=2, space="PSUM"))
        p_dram = ctx.enter_context(tc.tile_pool(name="dram", bufs=1, space="DRAM"))

        idxs = p_const.tile([128, NB * CMAX * 8], i16)
        nc.sync.dma_start(idxs[:], idxd.ap())
        ident = p_const.tile([128, 128], f32)
        nc.sync.dma_start(ident[:], identd.ap())
        bt = p_const.tile([128, 8 * N_LAYERS], f32)
        nc.sync.dma_start(bt[:], bTd.ap())

        hT = p_big.tile([128, 4, PADS], f32)     # resident h^T (fp32)
        ZT = p_big.tile([128, 4, PADS], bf16)    # (h + agg)^T, bf16 for MLP
        Y1T = p_big.tile([128, 4, PADS], bf16)   # hidden activation^T
        for kc in range(4):
            nc.sync.dma_start(hT[:, kc, :], xT.ap()[kc * 128:(kc + 1) * 128, :])

        h_shard = [p_dram.tile([PADS, D], f8, name=f"hsh{l}") for l in range(2)]
        ag_out = [p_dram.tile([PADN, D], f8, addr_space="Shared", name=f"ag{l}")
                  for l in range(2)]

        def emit_gather(l, b, gsrc):
            """One indirect gather for (layer l, block b): all C chunks."""
            C = C_list[b]
            g = p_g.tile([128, CMAX, D], f8, tag="g", name="g")
            nc.gpsimd.dma_gather(
                out_ap=g[:, :C, :],
                in_ap=gsrc,
                idxs_ap=idxs[:, b * CMAX * 8:b * CMAX * 8 + C * 8],
                num_idxs=C * 128,
                num_idxs_reg=C * 128,
                elem_size=D,
                single_packet=False,
                queue_num=0,
            )
            return g

        # Layer 0 gathers: source xg is an input, fire immediately.
        g_tiles = [emit_gather(0, b, xg.ap()) for b in range(NB)]

        # Small collective to absorb one-time ncfw/collective-stack startup
        # cost while layer 0 computes (emitted after the L0 gather preps so it
        # does not delay them in the GpSimd FIFO).
        wa_in = p_dram.tile([128, 64], bf16, name="wa_in")
        wa_out = p_dram.tile([128 * CORES, 64], bf16, addr_space="Shared", name="wa_out")
        nc.sync.dma_start(wa_in[:, :], identd.ap()[0:128, 0:32].bitcast(bf16)[:, 0:64])
        nc.gpsimd.collective_compute(
            "AllGather", mybir.AluOpType.bypass,
            replica_groups=[list(range(CORES))],
            ins=[wa_in.opt()], outs=[wa_out.opt()])

        for l in range(N_LAYERS):
            W0t = p_w.tile([128, 4, D], bf16, tag="w", name="W0t")
            W1t = p_w.tile([128, 4, D], bf16, tag="w", name="W1t")
            for kc in range(4):
                nc.sync.dma_start(W0t[:, kc, :], Wd.ap()[2 * l, kc * 128:(kc + 1) * 128, :])
                nc.sync.dma_start(W1t[:, kc, :], Wd.ap()[2 * l + 1, kc * 128:(kc + 1) * 128, :])

            # ---- aggregation: agg[node, feat] per 128-node dst block ----
            for b in range(NB):
                C = C_list[b]
                g = g_tiles[b]
                S_b = p_s.tile([128, CMAX, 128], f8, tag="s", name="S_b")
                nc.sync.dma_start(S_b[:, :C, :], Sd.ap()[b, :, :C * 128])

                ps = p_aggps.tile([128, D], f32, name="ps")
                for cc in range(C):
                    nc.tensor.matmul(ps[:], lhsT=S_b[:, cc, :], rhs=g[:, cc, :],
                                     start=(cc == 0), stop=(cc == C - 1))
                aggN = p_aggn.tile([128, D], f32, name="aggN")
                nc.vector.tensor_copy(aggN[:], ps[:])
                for fc in range(4):
                    pt = p_tps.tile([128, 128], f32, tag="t", name="pt")
                    nc.tensor.transpose(pt[:], aggN[:, fc * 128:(fc + 1) * 128], ident[:])
                    nc.vector.tensor_add(ZT[:, fc, b * 128:(b + 1) * 128], pt[:],
                                         hT[:, fc, b * 128:(b + 1) * 128])

            # ---- MLP (feature-major, bf16) ----
            for j in range(2):
                rhs_big = ZT if j == 0 else Y1T
                Wt = W0t if j == 0 else W1t
                for (nofs, nw) in NCHUNK:
                    for mc in range(4):
                        ps2 = p_mlpps.tile([128, D], f32, tag="mlp", name="ps2")
                        for kc in range(4):
                            nc.tensor.matmul(
                                ps2[:, :nw],
                                lhsT=Wt[:, kc, mc * 128:(mc + 1) * 128],
                                rhs=rhs_big[:, kc, nofs:nofs + nw],
                                start=(kc == 0), stop=(kc == 3))
                        col = (2 * l + j) * 4 + mc
                        bias = bt[:, col:col + 1]
                        if j == 0:
                            nc.scalar.activation(Y1T[:, mc, nofs:nofs + nw],
                                                 ps2[:, :nw], AF.Relu, bias=bias)
                        elif l < N_LAYERS - 1:
                            nc.scalar.activation(hT[:, mc, nofs:nofs + nw],
                                                 ps2[:, :nw], AF.Relu, bias=bias)
                        else:
                            ot = p_hbf.tile([128, 512], f32, tag="ot", name="ot")
                            nc.scalar.activation(ot[:, :nw], ps2[:, :nw],
                                                 AF.Identity, bias=bias)
                            nc.sync.dma_start(
                                outTd.ap()[mc * 128:(mc + 1) * 128, nofs:nofs + nw],
                                ot[:, :nw])

            if l < N_LAYERS - 1:
                # h^T -> node-major fp8 shard in DRAM, then AllGather.
                for b in range(NB):
                    hb = p_hbf.tile([128, D], f8, tag="hbf", name="hb")
                    for fc in range(4):
                        pt2 = p_tps.tile([128, 128], f32, tag="t", name="pt2")
                        nc.tensor.transpose(pt2[:], hT[:, fc, b * 128:(b + 1) * 128],
                                            ident[:])
                        nc.scalar.copy(hb[:, fc * 128:(fc + 1) * 128], pt2[:])
                    nc.sync.dma_start(h_shard[l][b * 128:(b + 1) * 128, :], hb[:])
                nc.gpsimd.collective_compute(
                    "AllGather",
                    mybir.AluOpType.bypass,
                    replica_groups=[list(range(CORES))],
                    ins=[h_shard[l].opt()],
                    outs=[ag_out[l].opt()],
                )
                # Next layer's gathers: emitted after the collective so Tile
                # records the AllGather output as their producer (RAW dep).
                g_tiles = [emit_gather(l + 1, b, ag_out[l][:, :]) for b in range(NB)]

    nc.compile()
    return nc


def kernel(**inputs):
    global LAST_RESULTS
    from concourse import bass_utils

    in_maps, C_list, CMAX = _prep_host(
        inputs["x"], inputs["edge_index"], inputs["Ws"], inputs["bs"])
    nc = build_program(C_list, CMAX)
    res = bass_utils.run_bass_kernel_spmd(
        nc, in_maps, core_ids=list(range(CORES)),
        trace=bool(int(os.environ.get("GIN_TRACE", "0"))),
        tmpdir=os.environ.get("GIN_TMPDIR"),
    )
    LAST_RESULTS = res
    out = np.empty((N_NODES, D), np.float32)
    for c in range(CORES):
        out[c * SHARD:(c + 1) * SHARD] = res.results[c]["outT"][:, :SHARD].T
    return out


# revision 12
# speedup vs baseline: 1.3134x; 1.3134x over previous
"""GIN (3-layer) Trainium2 Bass kernel, 8-core SPMD.

Sharding: nodes (and their incident edges, by dst) are partitioned across the
8 cores; segment_sum is computed locally per dst shard; node features are
exchanged between layers with an AllGather; MLP weights are replicated.

v2 (fp8 + prepared gathers):
  - the gather path (x / h rows pulled per edge, and the one-hot selector S)
    runs in fp8e4 (TRN e4m3, max 240): halves HBM gather traffic, selector
    traffic, and the inter-layer AllGather size. MLP stays bf16, residual h
    stays fp32.
  - indirect gathers use SWDGE prepare_only + trigger_dma: descriptor
    generation for layer l+1's gathers runs on the Q7 during layer l's
    compute; the trigger (which carries the RAW dep on the AllGather output)
    fires them the moment the AllGather lands. One gather per dst block.
  - agg matmul: per 128-edge chunk, the fp8 one-hot selector S is the
    stationary operand, gathered fp8 rows are moving; chunks accumulate in
    PSUM -> agg[node, feat]; transposed on the PE and added to resident fp32
    h^T. The 2-layer MLP runs feature-major in bf16 with fused bias+ReLU on
    the scalar engine.
"""

import os
import sys
from contextlib import ExitStack

import numpy as np

for _p in ("/opt/trn_rl_repo", "/root/.axon_site/_ro/trn_rl_repo"):
    if os.path.isdir(_p) and _p not in sys.path:
        sys.path.append(_p)

import ml_dtypes

N_NODES = 10000
N_EDGES = 160000
D = 512
N_LAYERS = 3
CORES = 8
SHARD = N_NODES // CORES          # 1250 nodes per core
PADS = 1280                       # padded shard (multiple of 128)
PADN = CORES * PADS               # padded full node count (10240)
NB = PADS // 128                  # dst blocks per core (10)

BF16 = ml_dtypes.bfloat16
F8 = ml_dtypes.float8_e4m3        # TRN fp8e4 (e4m3, max normal 240)

# Results of the last kernel() call (BassKernelResults) for the test harness.
LAST_RESULTS = None


def _prep_host(x, edge_index, Ws, bs):
    """Per-core input maps + per-block chunk counts (uniform across cores)."""
    x = np.asarray(x, np.float32)
    src = np.asarray(edge_index[0], np.int64)
    dst = np.asarray(edge_index[1], np.int64)
    Ws = np.asarray(Ws, np.float32)
    bs = np.asarray(bs, np.float32)

    # Padded gather row index for every edge's source node.
    gidx_all = (src // SHARD) * PADS + (src % SHARD)

    owner = dst // SHARD
    li = dst % SHARD
    blk = li // 128
    slot = li - blk * 128

    # Per (core, block) unique-src counts (post-dedup) set the chunk counts.
    key = (owner * NB + blk) * PADN + gidx_all
    ucnt = np.zeros(CORES * NB, np.int64)
    kb = np.unique(key) // PADN
    np.add.at(ucnt, kb, 1)
    ucnt = ucnt.reshape(CORES, NB)
    C_list = [max(1, int(-(-ucnt[:, b].max() // 128))) for b in range(NB)]
    CMAX = max(C_list)

    # Full padded x in fp8 (gather source for layer 0), shared by all cores.
    xg_pad = np.zeros((PADN, D), F8)
    for o in range(CORES):
        xg_pad[o * PADS:o * PADS + SHARD] = x[o * SHARD:(o + 1) * SHARD].astype(F8)

    Wd = np.ascontiguousarray(Ws.reshape(2 * N_LAYERS, D, D).astype(BF16))
    bT = np.ascontiguousarray(
        bs.reshape(2 * N_LAYERS, 4, 128).transpose(2, 0, 1).reshape(128, 8 * N_LAYERS))
    ident = np.eye(128, dtype=np.float32)

    order = np.lexsort((blk, owner))  # edges grouped by (owner, block)
    e_sorted = order
    bounds = np.searchsorted(owner[order] * NB + blk[order], np.arange(CORES * NB + 1))

    in_maps = []
    for c in range(CORES):
        Sd = np.zeros((NB, 128, CMAX * 128), F8)
        idxd = np.zeros((128, NB * CMAX * 8), np.int16)
        for b in range(NB):
            C = C_list[b]
            lo, hi = bounds[c * NB + b], bounds[c * NB + b + 1]
            e = e_sorted[lo:hi]
            # Deduplicate src nodes within the block; S carries multiplicity.
            uniq, inv = np.unique(gidx_all[e], return_inverse=True)
            n = len(uniq)
            glist = np.zeros(C * 128, np.int16)
            glist[:n] = uniq.astype(np.int16)
            np.add.at(Sd[b], (inv % 128, (inv // 128) * 128 + slot[e]), 1.0)
            w = glist.reshape(C * 8, 16).T  # w[p, s] = glist[s*16 + p]
            idxd[:, b * CMAX * 8:b * CMAX * 8 + C * 8] = np.tile(w, (8, 1))
        xT_own = np.zeros((D, PADS), np.float32)
        xT_own[:, :SHARD] = x[c * SHARD:(c + 1) * SHARD].T
        in_maps.append({
            "xg": xg_pad,
            "xT": xT_own,
            "Wd": Wd,
            "bT": bT,
            "ident": ident,
            "Sd": Sd,
            "idxd": idxd,
        })
    return in_maps, C_list, CMAX


def build_program(C_list, CMAX):
    import concourse.bacc as bacc
    import concourse.bass as bass
    import concourse.mybir as mybir
    import concourse.tile as tile

    dt = mybir.dt
    f32, bf16, i16, f8 = dt.float32, dt.bfloat16, dt.int16, dt.float8e4
    AF = mybir.ActivationFunctionType

    nc = bacc.Bacc("TRN2", target_bir_lowering=False, debug=False,
                   enable_asserts=False, num_devices=CORES, num_swdge_queues=4)

    xg = nc.dram_tensor("xg", [PADN, D], f8, kind="ExternalInput")
    xT = nc.dram_tensor("xT", [D, PADS], f32, kind="ExternalInput")
    Wd = nc.dram_tensor("Wd", [2 * N_LAYERS, D, D], bf16, kind="ExternalInput")
    bTd = nc.dram_tensor("bT", [128, 8 * N_LAYERS], f32, kind="ExternalInput")
    identd = nc.dram_tensor("ident", [128, 128], f32, kind="ExternalInput")
    Sd = nc.dram_tensor("Sd", [NB, 128, CMAX * 128], f8, kind="ExternalInput")
    idxd = nc.dram_tensor("idxd", [128, NB * CMAX * 8], i16, kind="ExternalInput")
    outTd = nc.dram_tensor("outT", [D, PADS], f32, kind="ExternalOutput")

    NCHUNK = [(0, 512), (512, 512), (1024, PADS - 1024)]  # node-dim tiles for MLP

    with tile.TileContext(nc) as tc, ExitStack() as ctx:
        p_const = ctx.enter_context(tc.tile_pool(name="const", bufs=1))
        p_big = ctx.enter_context(tc.tile_pool(name="big", bufs=1))
        p_g = ctx.enter_context(tc.tile_pool(name="gth", bufs=NB))
        p_s = ctx.enter_context(tc.tile_pool(name="sel", bufs=4))
        p_aggn = ctx.enter_context(tc.tile_pool(name="aggn", bufs=3))
        p_w = ctx.enter_context(tc.tile_pool(name="wts", bufs=2))
        p_hbf = ctx.enter_context(tc.tile_pool(name="hbf", bufs=2))
        p_aggps = ctx.enter_context(tc.tile_pool(name="aggps", bufs=2, space="PSUM"))
        p_tps = ctx.enter_context(tc.tile_pool(name="tps", bufs=4, space="PSUM"))
        p_mlpps = ctx.enter_context(tc.tile_pool(name="mlpps", bufs=2, space="PSUM"))
        p_dram = ctx.enter_context(tc.tile_pool(name="dram", bufs=1, space="DRAM"))

        idxs = p_const.tile([128, NB * CMAX * 8], i16)
        nc.sync.dma_start(idxs[:], idxd.ap())
        ident = p_const.tile([128, 128], f32)
        nc.sync.dma_start(ident[:], identd.ap())
        bt = p_const.tile([128, 8 * N_LAYERS], f32)
        nc.sync.dma_start(bt[:], bTd.ap())

        hT = p_big.tile([128, 4, PADS], f32)     # resident h^T (fp32)
        ZT = p_big.tile([128, 4, PADS], bf16)    # (h + agg)^T, bf16 for MLP
        Y1T = p_big.tile([128, 4, PADS], bf16)   # hidden activation^T
        for kc in range(4):
            nc.sync.dma_start(hT[:, kc, :], xT.ap()[kc * 128:(kc + 1) * 128, :])

        h_shard = [p_dram.tile([PADS, D], f8, name=f"hsh{l}") for l in range(2)]
        ag_out = [p_dram.tile([PADN, D], f8, addr_space="Shared", name=f"ag{l}")
                  for l in range(2)]

        def emit_gather(l, b, gsrc):
            """One indirect gather for (layer l, block b): all C chunks.

            queue_num tracks the pool-DMA ordinal so Tile's DMASW lane
            rotation (mod 8) stays consistent with the queue (mod 4): lane L
            always fires from queue L%4 (the sim enforces this lane<->queue
            lock, and a mismatch costs pessimistic semaphore waits on HW).
            """
            C = C_list[b]
            g = p_g.tile([128, CMAX, D], f8, tag="g", name="g")
            nc.gpsimd.dma_gather(
                out_ap=g[:, :C, :],
                in_ap=gsrc,
                idxs_ap=idxs[:, b * CMAX * 8:b * CMAX * 8 + C * 8],
                num_idxs=C * 128,
                num_idxs_reg=C * 128,
                elem_size=D,
                single_packet=False,
                queue_num=(NB * l + b) % 4,
            )
            return g

        # Layer 0 gathers: source xg is an input, fire immediately.
        g_tiles = [emit_gather(0, b, xg.ap()) for b in range(NB)]

        # Small collective to absorb one-time ncfw/collective-stack startup
        # cost while layer 0 computes (emitted after the L0 gather preps so it
        # does not delay them in the GpSimd FIFO).
        wa_in = p_dram.tile([128, 64], bf16, name="wa_in")
        wa_out = p_dram.tile([128 * CORES, 64], bf16, addr_space="Shared", name="wa_out")
        nc.sync.dma_start(wa_in[:, :], identd.ap()[0:128, 0:32].bitcast(bf16)[:, 0:64])
        nc.gpsimd.collective_compute(
            "AllGather", mybir.AluOpType.bypass,
            replica_groups=[list(range(CORES))],
            ins=[wa_in.opt()], outs=[wa_out.opt()])

        for l in range(N_LAYERS):
            W0t = p_w.tile([128, 4, D], bf16, tag="w", name="W0t")
            W1t = p_w.tile([128, 4, D], bf16, tag="w", name="W1t")
            for kc in range(4):
                nc.sync.dma_start(W0t[:, kc, :], Wd.ap()[2 * l, kc * 128:(kc + 1) * 128, :])
                nc.sync.dma_start(W1t[:, kc, :], Wd.ap()[2 * l + 1, kc * 128:(kc + 1) * 128, :])

            # ---- aggregation: agg[node, feat] per 128-node dst block ----
            for b in range(NB):
                C = C_list[b]
                g = g_tiles[b]
                S_b = p_s.tile([128, CMAX, 128], f8, tag="s", name="S_b")
                nc.sync.dma_start(S_b[:, :C, :], Sd.ap()[b, :, :C * 128])

                ps = p_aggps.tile([128, D], f32, name="ps")
                for cc in range(C):
                    nc.tensor.matmul(ps[:], lhsT=S_b[:, cc, :], rhs=g[:, cc, :],
                                     start=(cc == 0), stop=(cc == C - 1))
                aggN = p_aggn.tile([128, D], f32, name="aggN")
                nc.vector.tensor_copy(aggN[:], ps[:])
                for fc in range(4):
                    pt = p_tps.tile([128, 128], f32, tag="t", name="pt")
                    nc.tensor.transpose(pt[:], aggN[:, fc * 128:(fc + 1) * 128], ident[:])
                    nc.vector.tensor_add(ZT[:, fc, b * 128:(b + 1) * 128], pt[:],
                                         hT[:, fc, b * 128:(b + 1) * 128])

            # ---- MLP (feature-major, bf16) ----
            for j in range(2):
                rhs_big = ZT if j == 0 else Y1T
                Wt = W0t if j == 0 else W1t
                for (nofs, nw) in NCHUNK:
                    for mc in range(4):
                        ps2 = p_mlpps.tile([128, D], f32, tag="mlp", name="ps2")
                        for kc in range(4):
                            nc.tensor.matmul(
                                ps2[:, :nw],
                                lhsT=Wt[:, kc, mc * 128:(mc + 1) * 128],
                                rhs=rhs_big[:, kc, nofs:nofs + nw],
                                start=(kc == 0), stop=(kc == 3))
                        col = (2 * l + j) * 4 + mc
                        bias = bt[:, col:col + 1]
                        if j == 0:
                            nc.scalar.activation(Y1T[:, mc, nofs:nofs + nw],
                                                 ps2[:, :nw], AF.Relu, bias=bias)
                        elif l < N_LAYERS - 1:
                            nc.scalar.activation(hT[:, mc, nofs:nofs + nw],
                                                 ps2[:, :nw], AF.Relu, bias=bias)
                        else:
                            ot = p_hbf.tile([128, 512], f32, tag="ot", name="ot")
                            nc.scalar.activation(ot[:, :nw], ps2[:, :nw],
                                                 AF.Identity, bias=bias)
                            nc.sync.dma_start(
                                outTd.ap()[mc * 128:(mc + 1) * 128, nofs:nofs + nw],
                                ot[:, :nw])

            if l < N_LAYERS - 1:
                # h^T -> node-major fp8 shard in DRAM, then AllGather.
                for b in range(NB):
                    hb = p_hbf.tile([128, D], f8, tag="hbf", name="hb")
                    for fc in range(4):
                        pt2 = p_tps.tile([128, 128], f32, tag="t", name="pt2")
                        nc.tensor.transpose(pt2[:], hT[:, fc, b * 128:(b + 1) * 128],
                                            ident[:])
                        nc.scalar.copy(hb[:, fc * 128:(fc + 1) * 128], pt2[:])
                    nc.sync.dma_start(h_shard[l][b * 128:(b + 1) * 128, :], hb[:])
                nc.gpsimd.collective_compute(
                    "AllGather",
                    mybir.AluOpType.bypass,
                    replica_groups=[list(range(CORES))],
                    ins=[h_shard[l].opt()],
                    outs=[ag_out[l].opt()],
                )
                # Next layer's gathers: emitted after the collective so Tile
                # records the AllGather output as their producer (RAW dep).
                g_tiles = [emit_gather(l + 1, b, ag_out[l][:, :]) for b in range(NB)]

    nc.compile()
    return nc


def kernel(**inputs):
    global LAST_RESULTS
    from concourse import bass_utils

    in_maps, C_list, CMAX = _prep_host(
        inputs["x"], inputs["edge_index"], inputs["Ws"], inputs["bs"])
    nc = build_program(C_list, CMAX)
    res = bass_utils.run_bass_kernel_spmd(
        nc, in_maps, core_ids=list(range(CORES)),
        trace=bool(int(os.environ.get("GIN_TRACE", "0"))),
        tmpdir=os.environ.get("GIN_TMPDIR"),
    )
    LAST_RESULTS = res
    out = np.empty((N_NODES, D), np.float32)
    for c in range(CORES):
        out[c * SHARD:(c + 1) * SHARD] = res.results[c]["outT"][:, :SHARD].T
    return out


# revision 15
# speedup vs baseline: 1.3511x; 1.0287x over previous
"""GIN (3-layer) Trainium2 Bass kernel, 8-core SPMD.

Sharding: nodes (and their incident edges, by dst) are partitioned across the
8 cores; segment_sum is computed locally per dst shard; node features are
exchanged between layers with an AllGather; MLP weights are replicated.

v2 (fp8 + prepared gathers):
  - the gather path (x / h rows pulled per edge, and the one-hot selector S)
    runs in fp8e4 (TRN e4m3, max 240): halves HBM gather traffic, selector
    traffic, and the inter-layer AllGather size. MLP stays bf16, residual h
    stays fp32.
  - indirect gathers use SWDGE prepare_only + trigger_dma: descriptor
    generation for layer l+1's gathers runs on the Q7 during layer l's
    compute; the trigger (which carries the RAW dep on the AllGather output)
    fires them the moment the AllGather lands. One gather per dst block.
  - agg matmul: per 128-edge chunk, the fp8 one-hot selector S is the
    stationary operand, gathered fp8 rows are moving; chunks accumulate in
    PSUM -> agg[node, feat]; transposed on the PE and added to resident fp32
    h^T. The 2-layer MLP runs feature-major in bf16 with fused bias+ReLU on
    the scalar engine.
"""

import os
import sys
from contextlib import ExitStack

import numpy as np

for _p in ("/opt/trn_rl_repo", "/root/.axon_site/_ro/trn_rl_repo"):
    if os.path.isdir(_p) and _p not in sys.path:
        sys.path.append(_p)

import ml_dtypes

N_NODES = 10000
N_EDGES = 160000
D = 512
N_LAYERS = 3
CORES = 8
SHARD = N_NODES // CORES          # 1250 nodes per core
PADS = 1280                       # padded shard (multiple of 128)
PADN = CORES * PADS               # padded full node count (10240)
NB = PADS // 128                  # dst blocks per core (10)

BF16 = ml_dtypes.bfloat16
F8 = ml_dtypes.float8_e4m3        # TRN fp8e4 (e4m3, max normal 240)

# Results of the last kernel() call (BassKernelResults) for the test harness.
LAST_RESULTS = None


def _prep_host(x, edge_index, Ws, bs):
    """Per-core input maps + per-block chunk counts (uniform across cores)."""
    x = np.asarray(x, np.float32)
    src = np.asarray(edge_index[0], np.int64)
    dst = np.asarray(edge_index[1], np.int64)
    Ws = np.asarray(Ws, np.float32)
    bs = np.asarray(bs, np.float32)

    # Padded gather row index for every edge's source node.
    gidx_all = (src // SHARD) * PADS + (src % SHARD)

    owner = dst // SHARD
    li = dst % SHARD
    blk = li // 128
    slot = li - blk * 128

    # Per (core, block) unique-src counts (post-dedup) set the chunk counts.
    key = (owner * NB + blk) * PADN + gidx_all
    ucnt = np.zeros(CORES * NB, np.int64)
    kb = np.unique(key) // PADN
    np.add.at(ucnt, kb, 1)
    ucnt = ucnt.reshape(CORES, NB)
    C_list = [max(1, int(-(-ucnt[:, b].max() // 128))) for b in range(NB)]
    CMAX = max(C_list)

    # Full padded x in fp8 (gather source for layer 0), shared by all cores.
    xg_pad = np.zeros((PADN, D), F8)
    for o in range(CORES):
        xg_pad[o * PADS:o * PADS + SHARD] = x[o * SHARD:(o + 1) * SHARD].astype(F8)

    Wd = np.ascontiguousarray(Ws.reshape(2 * N_LAYERS, D, D).astype(BF16))
    bT = np.ascontiguousarray(
        bs.reshape(2 * N_LAYERS, 4, 128).transpose(2, 0, 1).reshape(128, 8 * N_LAYERS))
    ident = np.eye(128, dtype=np.float32)

    order = np.lexsort((blk, owner))  # edges grouped by (owner, block)
    e_sorted = order
    bounds = np.searchsorted(owner[order] * NB + blk[order], np.arange(CORES * NB + 1))

    in_maps = []
    for c in range(CORES):
        Sd = np.zeros((NB, 128, CMAX * 128), F8)
        idxd = np.zeros((128, NB * CMAX * 8), np.int16)
        for b in range(NB):
            C = C_list[b]
            lo, hi = bounds[c * NB + b], bounds[c * NB + b + 1]
            e = e_sorted[lo:hi]
            # Deduplicate src nodes within the block; S carries multiplicity.
            uniq, inv = np.unique(gidx_all[e], return_inverse=True)
            n = len(uniq)
            glist = np.zeros(C * 128, np.int16)
            glist[:n] = uniq.astype(np.int16)
            np.add.at(Sd[b], (inv % 128, (inv // 128) * 128 + slot[e]), 1.0)
            w = glist.reshape(C * 8, 16).T  # w[p, s] = glist[s*16 + p]
            idxd[:, b * CMAX * 8:b * CMAX * 8 + C * 8] = np.tile(w, (8, 1))
        xT_own = np.zeros((D, PADS), np.float32)
        xT_own[:, :SHARD] = x[c * SHARD:(c + 1) * SHARD].T
        in_maps.append({
            "xg": xg_pad,
            "xT": xT_own,
            "Wd": Wd,
            "bT": bT,
            "ident": ident,
            "Sd": Sd,
            "idxd": idxd,
        })
    return in_maps, C_list, CMAX


def build_program(C_list, CMAX):
    import concourse.bacc as bacc
    import concourse.bass as bass
    import concourse.mybir as mybir
    import concourse.tile as tile

    dt = mybir.dt
    f32, bf16, i16, f8 = dt.float32, dt.bfloat16, dt.int16, dt.float8e4
    AF = mybir.ActivationFunctionType

    nc = bacc.Bacc("TRN2", target_bir_lowering=False, debug=False,
                   enable_asserts=False, num_devices=CORES, num_swdge_queues=4)

    xg = nc.dram_tensor("xg", [PADN, D], f8, kind="ExternalInput")
    xT = nc.dram_tensor("xT", [D, PADS], f32, kind="ExternalInput")
    Wd = nc.dram_tensor("Wd", [2 * N_LAYERS, D, D], bf16, kind="ExternalInput")
    bTd = nc.dram_tensor("bT", [128, 8 * N_LAYERS], f32, kind="ExternalInput")
    identd = nc.dram_tensor("ident", [128, 128], f32, kind="ExternalInput")
    Sd = nc.dram_tensor("Sd", [NB, 128, CMAX * 128], f8, kind="ExternalInput")
    idxd = nc.dram_tensor("idxd", [128, NB * CMAX * 8], i16, kind="ExternalInput")
    outTd = nc.dram_tensor("outT", [D, PADS], f32, kind="ExternalOutput")

    NCHUNK = [(0, 512), (512, 512), (1024, PADS - 1024)]  # node-dim tiles for MLP

    with tile.TileContext(nc) as tc, ExitStack() as ctx:
        p_const = ctx.enter_context(tc.tile_pool(name="const", bufs=1))
        p_big = ctx.enter_context(tc.tile_pool(name="big", bufs=1))
        p_g = ctx.enter_context(tc.tile_pool(name="gth", bufs=NB))
        p_s = ctx.enter_context(tc.tile_pool(name="sel", bufs=6))
        p_aggn = ctx.enter_context(tc.tile_pool(name="aggn", bufs=3))
        p_w = ctx.enter_context(tc.tile_pool(name="wts", bufs=2))
        p_hbf = ctx.enter_context(tc.tile_pool(name="hbf", bufs=2))
        p_aggps = ctx.enter_context(tc.tile_pool(name="aggps", bufs=2, space="PSUM"))
        p_tps = ctx.enter_context(tc.tile_pool(name="tps", bufs=2, space="PSUM"))
        p_mlpps = ctx.enter_context(tc.tile_pool(name="mlpps", bufs=4, space="PSUM"))
        p_dram = ctx.enter_context(tc.tile_pool(name="dram", bufs=1, space="DRAM"))

        idxs = p_const.tile([128, NB * CMAX * 8], i16)
        nc.sync.dma_start(idxs[:], idxd.ap())
        ident = p_const.tile([128, 128], f32)
        nc.sync.dma_start(ident[:], identd.ap())
        bt = p_const.tile([128, 8 * N_LAYERS], f32)
        nc.sync.dma_start(bt[:], bTd.ap())

        hT = p_big.tile([128, 4, PADS], f32)     # resident h^T (fp32)
        ZT = p_big.tile([128, 4, PADS], bf16)    # (h + agg)^T, bf16 for MLP
        Y1T = p_big.tile([128, 4, PADS], bf16)   # hidden activation^T
        for kc in range(4):
            nc.sync.dma_start(hT[:, kc, :], xT.ap()[kc * 128:(kc + 1) * 128, :])

        h_shard = [p_dram.tile([PADS, D], f8, name=f"hsh{l}") for l in range(2)]
        ag_out = [p_dram.tile([PADN, D], f8, addr_space="Shared", name=f"ag{l}")
                  for l in range(2)]

        def emit_gather(l, b, gsrc):
            """One indirect gather for (layer l, block b): all C chunks.

            queue_num tracks the pool-DMA ordinal so Tile's DMASW lane
            rotation (mod 8) stays consistent with the queue (mod 4): lane L
            always fires from queue L%4 (the sim enforces this lane<->queue
            lock, and a mismatch costs pessimistic semaphore waits on HW).
            """
            C = C_list[b]
            g = p_g.tile([128, CMAX, D], f8, tag="g", name="g")
            nc.gpsimd.dma_gather(
                out_ap=g[:, :C, :],
                in_ap=gsrc,
                idxs_ap=idxs[:, b * CMAX * 8:b * CMAX * 8 + C * 8],
                num_idxs=C * 128,
                num_idxs_reg=C * 128,
                elem_size=D,
                single_packet=False,
                queue_num=(NB * l + b) % 4,
            )
            return g

        # Layer 0 gathers: source xg is an input, fire immediately.
        g_tiles = [emit_gather(0, b, xg.ap()) for b in range(NB)]

        # Small collective to absorb one-time ncfw/collective-stack startup
        # cost while layer 0 computes (emitted after the L0 gather preps so it
        # does not delay them in the GpSimd FIFO).
        wa_in = p_dram.tile([128, 64], bf16, name="wa_in")
        wa_out = p_dram.tile([128 * CORES, 64], bf16, addr_space="Shared", name="wa_out")
        nc.sync.dma_start(wa_in[:, :], identd.ap()[0:128, 0:32].bitcast(bf16)[:, 0:64])
        nc.gpsimd.collective_compute(
            "AllGather", mybir.AluOpType.bypass,
            replica_groups=[list(range(CORES))],
            ins=[wa_in.opt()], outs=[wa_out.opt()])

        for l in range(N_LAYERS):
            W0t = p_w.tile([128, 4, D], bf16, tag="w", name="W0t")
            W1t = p_w.tile([128, 4, D], bf16, tag="w", name="W1t")
            for kc in range(4):
                nc.sync.dma_start(W0t[:, kc, :], Wd.ap()[2 * l, kc * 128:(kc + 1) * 128, :])
                nc.sync.dma_start(W1t[:, kc, :], Wd.ap()[2 * l + 1, kc * 128:(kc + 1) * 128, :])

            # ---- aggregation: agg[node, feat] per 128-node dst block ----
            for b in range(NB):
                C = C_list[b]
                g = g_tiles[b]
                S_b = p_s.tile([128, CMAX, 128], f8, tag="s", name="S_b")
                nc.sync.dma_start(S_b[:, :C, :], Sd.ap()[b, :, :C * 128])

                ps = p_aggps.tile([128, D], f32, name="ps")
                for cc in range(C):
                    nc.tensor.matmul(ps[:], lhsT=S_b[:, cc, :], rhs=g[:, cc, :],
                                     start=(cc == 0), stop=(cc == C - 1))
                aggN = p_aggn.tile([128, D], f32, name="aggN")
                nc.vector.tensor_copy(aggN[:], ps[:])
                for fc in range(4):
                    pt = p_tps.tile([128, 128], f32, tag="t", name="pt")
                    nc.tensor.transpose(pt[:], aggN[:, fc * 128:(fc + 1) * 128], ident[:])
                    nc.vector.tensor_add(ZT[:, fc, b * 128:(b + 1) * 128], pt[:],
                                         hT[:, fc, b * 128:(b + 1) * 128])

            # ---- MLP (feature-major, bf16) ----
            for j in range(2):
                rhs_big = ZT if j == 0 else Y1T
                Wt = W0t if j == 0 else W1t
                for (nofs, nw) in NCHUNK:
                    for mc in range(4):
                        ps2 = p_mlpps.tile([128, D], f32, tag="mlp", name="ps2")
                        for kc in range(4):
                            nc.tensor.matmul(
                                ps2[:, :nw],
                                lhsT=Wt[:, kc, mc * 128:(mc + 1) * 128],
                                rhs=rhs_big[:, kc, nofs:nofs + nw],
                                start=(kc == 0), stop=(kc == 3))
                        col = (2 * l + j) * 4 + mc
                        bias = bt[:, col:col + 1]
                        if j == 0:
                            nc.scalar.activation(Y1T[:, mc, nofs:nofs + nw],
                                                 ps2[:, :nw], AF.Relu, bias=bias)
                        elif l < N_LAYERS - 1:
                            nc.scalar.activation(hT[:, mc, nofs:nofs + nw],
                                                 ps2[:, :nw], AF.Relu, bias=bias)
                        else:
                            ot = p_hbf.tile([128, 512], f32, tag="ot", name="ot")
                            nc.scalar.activation(ot[:, :nw], ps2[:, :nw],
                                                 AF.Identity, bias=bias)
                            nc.sync.dma_start(
                                outTd.ap()[mc * 128:(mc + 1) * 128, nofs:nofs + nw],
                                ot[:, :nw])

            if l < N_LAYERS - 1:
                # h^T -> node-major fp8 shard in DRAM, then AllGather.
                for b in range(NB):
                    hb = p_hbf.tile([128, D], f8, tag="hbf", name="hb")
                    for fc in range(4):
                        pt2 = p_tps.tile([128, 128], f32, tag="t", name="pt2")
                        nc.tensor.transpose(pt2[:], hT[:, fc, b * 128:(b + 1) * 128],
                                            ident[:])
                        nc.scalar.copy(hb[:, fc * 128:(fc + 1) * 128], pt2[:])
                    nc.sync.dma_start(h_shard[l][b * 128:(b + 1) * 128, :], hb[:])
                nc.gpsimd.collective_compute(
                    "AllGather",
                    mybir.AluOpType.bypass,
                    replica_groups=[list(range(CORES))],
                    ins=[h_shard[l].opt()],
                    outs=[ag_out[l].opt()],
                )
                # Next layer's gathers: emitted after the collective so Tile
                # records the AllGather output as their producer (RAW dep).
                g_tiles = [emit_gather(l + 1, b, ag_out[l][:, :]) for b in range(NB)]

    nc.compile()
    return nc


def kernel(**inputs):
    global LAST_RESULTS
    from concourse import bass_utils

    in_maps, C_list, CMAX = _prep_host(
        inputs["x"], inputs["edge_index"], inputs["Ws"], inputs["bs"])
    nc = build_program(C_list, CMAX)
    res = bass_utils.run_bass_kernel_spmd(
        nc, in_maps, core_ids=list(range(CORES)),
        trace=bool(int(os.environ.get("GIN_TRACE", "0"))),
        tmpdir=os.environ.get("GIN_TMPDIR"),
    )
    LAST_RESULTS = res
    out = np.empty((N_NODES, D), np.float32)
    for c in range(CORES):
        out[c * SHARD:(c + 1) * SHARD] = res.results[c]["outT"][:, :SHARD].T
    return out


# revision 16
# speedup vs baseline: 1.4196x; 1.0507x over previous
"""GIN (3-layer) Trainium2 Bass kernel, 8-core SPMD.

Sharding: nodes (and their incident edges, by dst) are partitioned across the
8 cores; segment_sum is computed locally per dst shard; node features are
exchanged between layers with an AllGather; MLP weights are replicated.

v2 (fp8 + prepared gathers):
  - the gather path (x / h rows pulled per edge, and the one-hot selector S)
    runs in fp8e4 (TRN e4m3, max 240): halves HBM gather traffic, selector
    traffic, and the inter-layer AllGather size. MLP stays bf16, residual h
    stays fp32.
  - indirect gathers use SWDGE prepare_only + trigger_dma: descriptor
    generation for layer l+1's gathers runs on the Q7 during layer l's
    compute; the trigger (which carries the RAW dep on the AllGather output)
    fires them the moment the AllGather lands. One gather per dst block.
  - agg matmul: per 128-edge chunk, the fp8 one-hot selector S is the
    stationary operand, gathered fp8 rows are moving; chunks accumulate in
    PSUM -> agg[node, feat]; transposed on the PE and added to resident fp32
    h^T. The 2-layer MLP runs feature-major in bf16 with fused bias+ReLU on
    the scalar engine.
"""

import os
import sys
from contextlib import ExitStack

import numpy as np

for _p in ("/opt/trn_rl_repo", "/root/.axon_site/_ro/trn_rl_repo"):
    if os.path.isdir(_p) and _p not in sys.path:
        sys.path.append(_p)

import ml_dtypes

N_NODES = 10000
N_EDGES = 160000
D = 512
N_LAYERS = 3
CORES = 8
SHARD = N_NODES // CORES          # 1250 nodes per core
PADS = 1280                       # padded shard (multiple of 128)
PADN = CORES * PADS               # padded full node count (10240)
NB = PADS // 128                  # dst blocks per core (10)

BF16 = ml_dtypes.bfloat16
F8 = ml_dtypes.float8_e4m3        # TRN fp8e4 (e4m3, max normal 240)

# Results of the last kernel() call (BassKernelResults) for the test harness.
LAST_RESULTS = None


def _prep_host(x, edge_index, Ws, bs):
    """Per-core input maps + per-block chunk counts (uniform across cores)."""
    x = np.asarray(x, np.float32)
    src = np.asarray(edge_index[0], np.int64)
    dst = np.asarray(edge_index[1], np.int64)
    Ws = np.asarray(Ws, np.float32)
    bs = np.asarray(bs, np.float32)

    # Padded gather row index for every edge's source node.
    gidx_all = (src // SHARD) * PADS + (src % SHARD)

    owner = dst // SHARD
    li = dst % SHARD
    blk = li // 128
    slot = li - blk * 128

    # Per (core, block) unique-src counts (post-dedup) set the chunk counts.
    key = (owner * NB + blk) * PADN + gidx_all
    ucnt = np.zeros(CORES * NB, np.int64)
    kb = np.unique(key) // PADN
    np.add.at(ucnt, kb, 1)
    ucnt = ucnt.reshape(CORES, NB)
    C_list = [max(1, int(-(-ucnt[:, b].max() // 128))) for b in range(NB)]
    CMAX = max(C_list)

    # Full padded x in fp8 (gather source for layer 0), shared by all cores.
    xg_pad = np.zeros((PADN, D), F8)
    for o in range(CORES):
        xg_pad[o * PADS:o * PADS + SHARD] = x[o * SHARD:(o + 1) * SHARD].astype(F8)

    Wd = np.ascontiguousarray(Ws.reshape(2 * N_LAYERS, D, D).astype(BF16))
    bT = np.ascontiguousarray(
        bs.reshape(2 * N_LAYERS, 4, 128).transpose(2, 0, 1).reshape(128, 8 * N_LAYERS))
    ident = np.eye(128, dtype=np.float32)

    order = np.lexsort((blk, owner))  # edges grouped by (owner, block)
    e_sorted = order
    bounds = np.searchsorted(owner[order] * NB + blk[order], np.arange(CORES * NB + 1))

    in_maps = []
    for c in range(CORES):
        Sd = np.zeros((NB, 128, CMAX * 128), F8)
        idxd = np.zeros((128, NB * CMAX * 8), np.int16)
        for b in range(NB):
            C = C_list[b]
            lo, hi = bounds[c * NB + b], bounds[c * NB + b + 1]
            e = e_sorted[lo:hi]
            # Deduplicate src nodes within the block; S carries multiplicity.
            uniq, inv = np.unique(gidx_all[e], return_inverse=True)
            n = len(uniq)
            glist = np.zeros(C * 128, np.int16)
            glist[:n] = uniq.astype(np.int16)
            np.add.at(Sd[b], (inv % 128, (inv // 128) * 128 + slot[e]), 1.0)
            w = glist.reshape(C * 8, 16).T  # w[p, s] = glist[s*16 + p]
            idxd[:, b * CMAX * 8:b * CMAX * 8 + C * 8] = np.tile(w, (8, 1))
        xT_own = np.zeros((D, PADS), np.float32)
        xT_own[:, :SHARD] = x[c * SHARD:(c + 1) * SHARD].T
        in_maps.append({
            "xg": xg_pad,
            "xT": xT_own,
            "Wd": Wd,
            "bT": bT,
            "ident": ident,
            "Sd": Sd,
            "idxd": idxd,
        })
    return in_maps, C_list, CMAX


def build_program(C_list, CMAX):
    import concourse.bacc as bacc
    import concourse.bass as bass
    import concourse.mybir as mybir
    import concourse.tile as tile

    dt = mybir.dt
    f32, bf16, i16, f8 = dt.float32, dt.bfloat16, dt.int16, dt.float8e4
    AF = mybir.ActivationFunctionType

    nc = bacc.Bacc("TRN2", target_bir_lowering=False, debug=False,
                   enable_asserts=False, num_devices=CORES, num_swdge_queues=4)

    xg = nc.dram_tensor("xg", [PADN, D], f8, kind="ExternalInput")
    xT = nc.dram_tensor("xT", [D, PADS], f32, kind="ExternalInput")
    Wd = nc.dram_tensor("Wd", [2 * N_LAYERS, D, D], bf16, kind="ExternalInput")
    bTd = nc.dram_tensor("bT", [128, 8 * N_LAYERS], f32, kind="ExternalInput")
    identd = nc.dram_tensor("ident", [128, 128], f32, kind="ExternalInput")
    Sd = nc.dram_tensor("Sd", [NB, 128, CMAX * 128], f8, kind="ExternalInput")
    idxd = nc.dram_tensor("idxd", [128, NB * CMAX * 8], i16, kind="ExternalInput")
    outTd = nc.dram_tensor("outT", [D, PADS], f32, kind="ExternalOutput")

    NCHUNK = [(0, 512), (512, 512), (1024, PADS - 1024)]  # node-dim tiles for MLP

    with tile.TileContext(nc) as tc, ExitStack() as ctx:
        p_const = ctx.enter_context(tc.tile_pool(name="const", bufs=1))
        p_big = ctx.enter_context(tc.tile_pool(name="big", bufs=1))
        p_g = ctx.enter_context(tc.tile_pool(name="gth", bufs=NB))
        p_s = ctx.enter_context(tc.tile_pool(name="sel", bufs=6))
        p_aggn = ctx.enter_context(tc.tile_pool(name="aggn", bufs=3))
        p_w = ctx.enter_context(tc.tile_pool(name="wts", bufs=2))
        p_hbf = ctx.enter_context(tc.tile_pool(name="hbf", bufs=2))
        p_aggps = ctx.enter_context(tc.tile_pool(name="aggps", bufs=2, space="PSUM"))
        p_tps = ctx.enter_context(tc.tile_pool(name="tps", bufs=2, space="PSUM"))
        p_mlpps = ctx.enter_context(tc.tile_pool(name="mlpps", bufs=4, space="PSUM"))
        p_dram = ctx.enter_context(tc.tile_pool(name="dram", bufs=1, space="DRAM"))

        idxs = p_const.tile([128, NB * CMAX * 8], i16)
        nc.sync.dma_start(idxs[:], idxd.ap())
        ident = p_const.tile([128, 128], f32)
        nc.sync.dma_start(ident[:], identd.ap())
        bt = p_const.tile([128, 8 * N_LAYERS], f32)
        nc.sync.dma_start(bt[:], bTd.ap())

        hT = p_big.tile([128, 4, PADS], f32)     # resident h^T (fp32)
        ZT = p_big.tile([128, 4, PADS], bf16)    # (h + agg)^T, bf16 for MLP
        Y1T = p_big.tile([128, 4, PADS], bf16)   # hidden activation^T
        for kc in range(4):
            nc.sync.dma_start(hT[:, kc, :], xT.ap()[kc * 128:(kc + 1) * 128, :])

        h_shard = [p_dram.tile([PADS, D], f8, name=f"hsh{l}") for l in range(2)]
        ag_out = [p_dram.tile([PADN, D], f8, addr_space="Shared", name=f"ag{l}")
                  for l in range(2)]

        def emit_gather(l, b, gsrc):
            """One indirect gather for (layer l, block b): all C chunks.

            queue_num tracks the pool-DMA ordinal so Tile's DMASW lane
            rotation (mod 8) stays consistent with the queue (mod 4): lane L
            always fires from queue L%4 (the sim enforces this lane<->queue
            lock, and a mismatch costs pessimistic semaphore waits on HW).
            """
            C = C_list[b]
            g = p_g.tile([128, CMAX, D], f8, tag="g", name="g")
            nc.gpsimd.dma_gather(
                out_ap=g[:, :C, :],
                in_ap=gsrc,
                idxs_ap=idxs[:, b * CMAX * 8:b * CMAX * 8 + C * 8],
                num_idxs=C * 128,
                num_idxs_reg=C * 128,
                elem_size=D,
                single_packet=False,
                queue_num=(NB * l + b) % 4,
            )
            return g

        # Layer 0 gathers: source xg is an input, fire immediately.
        g_tiles = [emit_gather(0, b, xg.ap()) for b in range(NB)]

        # Small collective to absorb one-time ncfw/collective-stack startup
        # cost while layer 0 computes (emitted after the L0 gather preps so it
        # does not delay them in the GpSimd FIFO).
        wa_in = p_dram.tile([128, 64], bf16, name="wa_in")
        wa_out = p_dram.tile([128 * CORES, 64], bf16, addr_space="Shared", name="wa_out")
        nc.sync.dma_start(wa_in[:, :], identd.ap()[0:128, 0:32].bitcast(bf16)[:, 0:64])
        nc.gpsimd.collective_compute(
            "AllGather", mybir.AluOpType.bypass,
            replica_groups=[list(range(CORES))],
            ins=[wa_in.opt()], outs=[wa_out.opt()])

        for l in range(N_LAYERS):
            W0t = p_w.tile([128, 4, D], bf16, tag="w", name="W0t")
            W1t = p_w.tile([128, 4, D], bf16, tag="w", name="W1t")
            for kc in range(4):
                nc.sync.dma_start(W0t[:, kc, :], Wd.ap()[2 * l, kc * 128:(kc + 1) * 128, :])
                nc.sync.dma_start(W1t[:, kc, :], Wd.ap()[2 * l + 1, kc * 128:(kc + 1) * 128, :])

            # ---- aggregation: agg[node, feat] per 128-node dst block ----
            for b in range(NB):
                C = C_list[b]
                g = g_tiles[b]
                S_b = p_s.tile([128, CMAX, 128], f8, tag="s", name="S_b")
                nc.sync.dma_start(S_b[:, :C, :], Sd.ap()[b, :, :C * 128])

                ps = p_aggps.tile([128, D], f32, name="ps")
                for cc in range(C):
                    nc.tensor.matmul(ps[:], lhsT=S_b[:, cc, :], rhs=g[:, cc, :],
                                     start=(cc == 0), stop=(cc == C - 1))
                aggN = p_aggn.tile([128, D], f32, name="aggN")
                nc.vector.tensor_copy(aggN[:], ps[:])
                for fc in range(4):
                    pt = p_tps.tile([128, 128], f32, tag="t", name="pt")
                    nc.tensor.transpose(pt[:], aggN[:, fc * 128:(fc + 1) * 128], ident[:])
                    nc.vector.tensor_add(ZT[:, fc, b * 128:(b + 1) * 128], pt[:],
                                         hT[:, fc, b * 128:(b + 1) * 128])

            # ---- MLP (feature-major, bf16), per node piece: j0 -> j1 ->
            # transpose-back + fp8 shard store. Pieces are independent, so
            # each piece's h rows reach DRAM while later pieces still compute
            # and the AllGather only waits on the last piece's store.
            for (nofs, nw) in NCHUNK:
                for j in range(2):
                    rhs_big = ZT if j == 0 else Y1T
                    Wt = W0t if j == 0 else W1t
                    for mc in range(4):
                        ps2 = p_mlpps.tile([128, D], f32, tag="mlp", name="ps2")
                        for kc in range(4):
                            nc.tensor.matmul(
                                ps2[:, :nw],
                                lhsT=Wt[:, kc, mc * 128:(mc + 1) * 128],
                                rhs=rhs_big[:, kc, nofs:nofs + nw],
                                start=(kc == 0), stop=(kc == 3))
                        col = (2 * l + j) * 4 + mc
                        bias = bt[:, col:col + 1]
                        if j == 0:
                            nc.scalar.activation(Y1T[:, mc, nofs:nofs + nw],
                                                 ps2[:, :nw], AF.Relu, bias=bias)
                        elif l < N_LAYERS - 1:
                            nc.scalar.activation(hT[:, mc, nofs:nofs + nw],
                                                 ps2[:, :nw], AF.Relu, bias=bias)
                        else:
                            ot = p_hbf.tile([128, 512], f32, tag="ot", name="ot")
                            nc.scalar.activation(ot[:, :nw], ps2[:, :nw],
                                                 AF.Identity, bias=bias)
                            nc.sync.dma_start(
                                outTd.ap()[mc * 128:(mc + 1) * 128, nofs:nofs + nw],
                                ot[:, :nw])
                if l < N_LAYERS - 1:
                    # h^T -> node-major fp8 rows for this piece's blocks.
                    for b in range(nofs // 128, (nofs + nw) // 128):
                        hb = p_hbf.tile([128, D], f8, tag="hbf", name="hb")
                        for fc in range(4):
                            pt2 = p_tps.tile([128, 128], f32, tag="t", name="pt2")
                            nc.tensor.transpose(pt2[:], hT[:, fc, b * 128:(b + 1) * 128],
                                                ident[:])
                            nc.scalar.copy(hb[:, fc * 128:(fc + 1) * 128], pt2[:])
                        nc.sync.dma_start(h_shard[l][b * 128:(b + 1) * 128, :], hb[:])

            if l < N_LAYERS - 1:
                nc.gpsimd.collective_compute(
                    "AllGather",
                    mybir.AluOpType.bypass,
                    replica_groups=[list(range(CORES))],
                    ins=[h_shard[l].opt()],
                    outs=[ag_out[l].opt()],
                )
                # Next layer's gathers: emitted after the collective so Tile
                # records the AllGather output as their producer (RAW dep).
                g_tiles = [emit_gather(l + 1, b, ag_out[l][:, :]) for b in range(NB)]

    nc.compile()
    return nc


def kernel(**inputs):
    global LAST_RESULTS
    from concourse import bass_utils

    in_maps, C_list, CMAX = _prep_host(
        inputs["x"], inputs["edge_index"], inputs["Ws"], inputs["bs"])
    nc = build_program(C_list, CMAX)
    res = bass_utils.run_bass_kernel_spmd(
        nc, in_maps, core_ids=list(range(CORES)),
        trace=bool(int(os.environ.get("GIN_TRACE", "0"))),
        tmpdir=os.environ.get("GIN_TMPDIR"),
    )
    LAST_RESULTS = res
    out = np.empty((N_NODES, D), np.float32)
    for c in range(CORES):
        out[c * SHARD:(c + 1) * SHARD] = res.results[c]["outT"][:, :SHARD].T
    return out


# revision 17
# speedup vs baseline: 1.4308x; 1.0079x over previous
"""GIN (3-layer) Trainium2 Bass kernel, 8-core SPMD.

Sharding: nodes (and their incident edges, by dst) are partitioned across the
8 cores; segment_sum is computed locally per dst shard; node features are
exchanged between layers with an AllGather; MLP weights are replicated.

v2 (fp8 + prepared gathers):
  - the gather path (x / h rows pulled per edge, and the one-hot selector S)
    runs in fp8e4 (TRN e4m3, max 240): halves HBM gather traffic, selector
    traffic, and the inter-layer AllGather size. MLP stays bf16, residual h
    stays fp32.
  - indirect gathers use SWDGE prepare_only + trigger_dma: descriptor
    generation for layer l+1's gathers runs on the Q7 during layer l's
    compute; the trigger (which carries the RAW dep on the AllGather output)
    fires them the moment the AllGather lands. One gather per dst block.
  - agg matmul: per 128-edge chunk, the fp8 one-hot selector S is the
    stationary operand, gathered fp8 rows are moving; chunks accumulate in
    PSUM -> agg[node, feat]; transposed on the PE and added to resident fp32
    h^T. The 2-layer MLP runs feature-major in bf16 with fused bias+ReLU on
    the scalar engine.
"""

import os
import sys
from contextlib import ExitStack

import numpy as np

for _p in ("/opt/trn_rl_repo", "/root/.axon_site/_ro/trn_rl_repo"):
    if os.path.isdir(_p) and _p not in sys.path:
        sys.path.append(_p)

import ml_dtypes

N_NODES = 10000
N_EDGES = 160000
D = 512
N_LAYERS = 3
CORES = 8
SHARD = N_NODES // CORES          # 1250 nodes per core
PADS = 1280                       # padded shard (multiple of 128)
PADN = CORES * PADS               # padded full node count (10240)
NB = PADS // 128                  # dst blocks per core (10)

BF16 = ml_dtypes.bfloat16
F8 = ml_dtypes.float8_e4m3        # TRN fp8e4 (e4m3, max normal 240)

# Results of the last kernel() call (BassKernelResults) for the test harness.
LAST_RESULTS = None


def _prep_host(x, edge_index, Ws, bs):
    """Per-core input maps + per-block chunk counts (uniform across cores)."""
    x = np.asarray(x, np.float32)
    src = np.asarray(edge_index[0], np.int64)
    dst = np.asarray(edge_index[1], np.int64)
    Ws = np.asarray(Ws, np.float32)
    bs = np.asarray(bs, np.float32)

    # Padded gather row index for every edge's source node.
    gidx_all = (src // SHARD) * PADS + (src % SHARD)

    owner = dst // SHARD
    li = dst % SHARD
    blk = li // 128
    slot = li - blk * 128

    # Per (core, block) unique-src counts (post-dedup) set the chunk counts.
    key = (owner * NB + blk) * PADN + gidx_all
    ucnt = np.zeros(CORES * NB, np.int64)
    kb = np.unique(key) // PADN
    np.add.at(ucnt, kb, 1)
    ucnt = ucnt.reshape(CORES, NB)
    C_list = [max(1, int(-(-ucnt[:, b].max() // 128))) for b in range(NB)]
    CMAX = max(C_list)

    # Full padded x in fp8 (gather source for layer 0), shared by all cores.
    xg_pad = np.zeros((PADN, D), F8)
    for o in range(CORES):
        xg_pad[o * PADS:o * PADS + SHARD] = x[o * SHARD:(o + 1) * SHARD].astype(F8)

    Wd = np.ascontiguousarray(Ws.reshape(2 * N_LAYERS, D, D).astype(BF16))
    bT = np.ascontiguousarray(
        bs.reshape(2 * N_LAYERS, 4, 128).transpose(2, 0, 1).reshape(128, 8 * N_LAYERS))
    ident = np.eye(128, dtype=np.float32)

    order = np.lexsort((blk, owner))  # edges grouped by (owner, block)
    e_sorted = order
    bounds = np.searchsorted(owner[order] * NB + blk[order], np.arange(CORES * NB + 1))

    in_maps = []
    for c in range(CORES):
        Sd = np.zeros((NB, 128, CMAX * 128), F8)
        idxd = np.zeros((128, NB * CMAX * 8), np.int16)
        for b in range(NB):
            C = C_list[b]
            lo, hi = bounds[c * NB + b], bounds[c * NB + b + 1]
            e = e_sorted[lo:hi]
            # Deduplicate src nodes within the block; S carries multiplicity.
            uniq, inv = np.unique(gidx_all[e], return_inverse=True)
            n = len(uniq)
            glist = np.zeros(C * 128, np.int16)
            glist[:n] = uniq.astype(np.int16)
            np.add.at(Sd[b], (inv % 128, (inv // 128) * 128 + slot[e]), 1.0)
            w = glist.reshape(C * 8, 16).T  # w[p, s] = glist[s*16 + p]
            idxd[:, b * CMAX * 8:b * CMAX * 8 + C * 8] = np.tile(w, (8, 1))
        xT_own = np.zeros((D, PADS), np.float32)
        xT_own[:, :SHARD] = x[c * SHARD:(c + 1) * SHARD].T
        in_maps.append({
            "xg": xg_pad,
            "xT": xT_own,
            "Wd": Wd,
            "bT": bT,
            "ident": ident,
            "Sd": Sd,
            "idxd": idxd,
        })
    return in_maps, C_list, CMAX


def build_program(C_list, CMAX):
    import concourse.bacc as bacc
    import concourse.bass as bass
    import concourse.mybir as mybir
    import concourse.tile as tile

    dt = mybir.dt
    f32, bf16, i16, f8 = dt.float32, dt.bfloat16, dt.int16, dt.float8e4
    AF = mybir.ActivationFunctionType

    nc = bacc.Bacc("TRN2", target_bir_lowering=False, debug=False,
                   enable_asserts=False, num_devices=CORES, num_swdge_queues=4)

    xg = nc.dram_tensor("xg", [PADN, D], f8, kind="ExternalInput")
    xT = nc.dram_tensor("xT", [D, PADS], f32, kind="ExternalInput")
    Wd = nc.dram_tensor("Wd", [2 * N_LAYERS, D, D], bf16, kind="ExternalInput")
    bTd = nc.dram_tensor("bT", [128, 8 * N_LAYERS], f32, kind="ExternalInput")
    identd = nc.dram_tensor("ident", [128, 128], f32, kind="ExternalInput")
    Sd = nc.dram_tensor("Sd", [NB, 128, CMAX * 128], f8, kind="ExternalInput")
    idxd = nc.dram_tensor("idxd", [128, NB * CMAX * 8], i16, kind="ExternalInput")
    outTd = nc.dram_tensor("outT", [D, PADS], f32, kind="ExternalOutput")

    NCHUNK = [(0, 512), (512, 512), (1024, PADS - 1024)]  # node-dim tiles for MLP

    with tile.TileContext(nc) as tc, ExitStack() as ctx:
        p_const = ctx.enter_context(tc.tile_pool(name="const", bufs=1))
        p_big = ctx.enter_context(tc.tile_pool(name="big", bufs=1))
        p_g = ctx.enter_context(tc.tile_pool(name="gth", bufs=NB))
        p_s = ctx.enter_context(tc.tile_pool(name="sel", bufs=6))
        p_aggn = ctx.enter_context(tc.tile_pool(name="aggn", bufs=3))
        p_w = ctx.enter_context(tc.tile_pool(name="wts", bufs=4))
        p_hbf = ctx.enter_context(tc.tile_pool(name="hbf", bufs=4))
        p_aggps = ctx.enter_context(tc.tile_pool(name="aggps", bufs=2, space="PSUM"))
        p_tps = ctx.enter_context(tc.tile_pool(name="tps", bufs=2, space="PSUM"))
        p_mlpps = ctx.enter_context(tc.tile_pool(name="mlpps", bufs=4, space="PSUM"))
        p_dram = ctx.enter_context(tc.tile_pool(name="dram", bufs=1, space="DRAM"))

        idxs = p_const.tile([128, NB * CMAX * 8], i16)
        nc.sync.dma_start(idxs[:], idxd.ap())
        ident = p_const.tile([128, 128], f32)
        nc.sync.dma_start(ident[:], identd.ap())
        bt = p_const.tile([128, 8 * N_LAYERS], f32)
        nc.sync.dma_start(bt[:], bTd.ap())

        hT = p_big.tile([128, 4, PADS], f32)     # resident h^T (fp32)
        ZT = p_big.tile([128, 4, PADS], bf16)    # (h + agg)^T, bf16 for MLP
        Y1T = p_big.tile([128, 4, PADS], bf16)   # hidden activation^T
        for kc in range(4):
            nc.sync.dma_start(hT[:, kc, :], xT.ap()[kc * 128:(kc + 1) * 128, :])

        h_shard = [p_dram.tile([PADS, D], f8, name=f"hsh{l}") for l in range(2)]
        ag_out = [p_dram.tile([PADN, D], f8, addr_space="Shared", name=f"ag{l}")
                  for l in range(2)]

        def emit_gather(l, b, gsrc):
            """One indirect gather for (layer l, block b): all C chunks.

            queue_num tracks the pool-DMA ordinal so Tile's DMASW lane
            rotation (mod 8) stays consistent with the queue (mod 4): lane L
            always fires from queue L%4 (the sim enforces this lane<->queue
            lock, and a mismatch costs pessimistic semaphore waits on HW).
            """
            C = C_list[b]
            g = p_g.tile([128, CMAX, D], f8, tag="g", name="g")
            nc.gpsimd.dma_gather(
                out_ap=g[:, :C, :],
                in_ap=gsrc,
                idxs_ap=idxs[:, b * CMAX * 8:b * CMAX * 8 + C * 8],
                num_idxs=C * 128,
                num_idxs_reg=C * 128,
                elem_size=D,
                single_packet=False,
                queue_num=(NB * l + b) % 4,
            )
            return g

        # Layer 0 gathers: source xg is an input, fire immediately.
        g_tiles = [emit_gather(0, b, xg.ap()) for b in range(NB)]

        # Small collective to absorb one-time ncfw/collective-stack startup
        # cost while layer 0 computes (emitted after the L0 gather preps so it
        # does not delay them in the GpSimd FIFO).
        wa_in = p_dram.tile([128, 64], bf16, name="wa_in")
        wa_out = p_dram.tile([128 * CORES, 64], bf16, addr_space="Shared", name="wa_out")
        nc.sync.dma_start(wa_in[:, :], identd.ap()[0:128, 0:32].bitcast(bf16)[:, 0:64])
        nc.gpsimd.collective_compute(
            "AllGather", mybir.AluOpType.bypass,
            replica_groups=[list(range(CORES))],
            ins=[wa_in.opt()], outs=[wa_out.opt()])

        for l in range(N_LAYERS):
            W0t = p_w.tile([128, 4, D], bf16, tag="w", name="W0t")
            W1t = p_w.tile([128, 4, D], bf16, tag="w", name="W1t")
            for kc in range(4):
                nc.sync.dma_start(W0t[:, kc, :], Wd.ap()[2 * l, kc * 128:(kc + 1) * 128, :])
                nc.sync.dma_start(W1t[:, kc, :], Wd.ap()[2 * l + 1, kc * 128:(kc + 1) * 128, :])

            # ---- aggregation: agg[node, feat] per 128-node dst block ----
            for b in range(NB):
                C = C_list[b]
                g = g_tiles[b]
                S_b = p_s.tile([128, CMAX, 128], f8, tag="s", name="S_b")
                nc.sync.dma_start(S_b[:, :C, :], Sd.ap()[b, :, :C * 128])

                ps = p_aggps.tile([128, D], f32, name="ps")
                for cc in range(C):
                    nc.tensor.matmul(ps[:], lhsT=S_b[:, cc, :], rhs=g[:, cc, :],
                                     start=(cc == 0), stop=(cc == C - 1))
                aggN = p_aggn.tile([128, D], f32, name="aggN")
                nc.vector.tensor_copy(aggN[:], ps[:])
                for fc in range(4):
                    pt = p_tps.tile([128, 128], f32, tag="t", name="pt")
                    nc.tensor.transpose(pt[:], aggN[:, fc * 128:(fc + 1) * 128], ident[:])
                    nc.vector.tensor_add(ZT[:, fc, b * 128:(b + 1) * 128], pt[:],
                                         hT[:, fc, b * 128:(b + 1) * 128])

            # ---- MLP (feature-major, bf16), per node piece: j0 -> j1 ->
            # transpose-back + fp8 shard store. Pieces are independent, so
            # each piece's h rows reach DRAM while later pieces still compute
            # and the AllGather only waits on the last piece's store.
            for (nofs, nw) in NCHUNK:
                for j in range(2):
                    rhs_big = ZT if j == 0 else Y1T
                    Wt = W0t if j == 0 else W1t
                    for mc in range(4):
                        ps2 = p_mlpps.tile([128, D], f32, tag="mlp", name="ps2")
                        for kc in range(4):
                            nc.tensor.matmul(
                                ps2[:, :nw],
                                lhsT=Wt[:, kc, mc * 128:(mc + 1) * 128],
                                rhs=rhs_big[:, kc, nofs:nofs + nw],
                                start=(kc == 0), stop=(kc == 3))
                        col = (2 * l + j) * 4 + mc
                        bias = bt[:, col:col + 1]
                        if j == 0:
                            nc.scalar.activation(Y1T[:, mc, nofs:nofs + nw],
                                                 ps2[:, :nw], AF.Relu, bias=bias)
                        elif l < N_LAYERS - 1:
                            nc.scalar.activation(hT[:, mc, nofs:nofs + nw],
                                                 ps2[:, :nw], AF.Relu, bias=bias)
                        else:
                            ot = p_hbf.tile([128, 512], f32, tag="ot", name="ot")
                            nc.scalar.activation(ot[:, :nw], ps2[:, :nw],
                                                 AF.Identity, bias=bias)
                            nc.sync.dma_start(
                                outTd.ap()[mc * 128:(mc + 1) * 128, nofs:nofs + nw],
                                ot[:, :nw])
                if l < N_LAYERS - 1:
                    # h^T -> node-major fp8 rows for this piece's blocks.
                    for b in range(nofs // 128, (nofs + nw) // 128):
                        hb = p_hbf.tile([128, D], f8, tag="hbf", name="hb")
                        for fc in range(4):
                            pt2 = p_tps.tile([128, 128], f32, tag="t", name="pt2")
                            nc.tensor.transpose(pt2[:], hT[:, fc, b * 128:(b + 1) * 128],
                                                ident[:])
                            nc.scalar.copy(hb[:, fc * 128:(fc + 1) * 128], pt2[:])
                        nc.sync.dma_start(h_shard[l][b * 128:(b + 1) * 128, :], hb[:])

            if l < N_LAYERS - 1:
                nc.gpsimd.collective_compute(
                    "AllGather",
                    mybir.AluOpType.bypass,
                    replica_groups=[list(range(CORES))],
                    ins=[h_shard[l].opt()],
                    outs=[ag_out[l].opt()],
                )
                # Next layer's gathers: emitted after the collective so Tile
                # records the AllGather output as their producer (RAW dep).
                g_tiles = [emit_gather(l + 1, b, ag_out[l][:, :]) for b in range(NB)]

    nc.compile()
    return nc


def kernel(**inputs):
    global LAST_RESULTS
    from concourse import bass_utils

    in_maps, C_list, CMAX = _prep_host(
        inputs["x"], inputs["edge_index"], inputs["Ws"], inputs["bs"])
    nc = build_program(C_list, CMAX)
    res = bass_utils.run_bass_kernel_spmd(
        nc, in_maps, core_ids=list(range(CORES)),
        trace=bool(int(os.environ.get("GIN_TRACE", "0"))),
        tmpdir=os.environ.get("GIN_TMPDIR"),
    )
    LAST_RESULTS = res
    out = np.empty((N_NODES, D), np.float32)
    for c in range(CORES):
        out[c * SHARD:(c + 1) * SHARD] = res.results[c]["outT"][:, :SHARD].T
    return out
